# revision 5
# baseline (speedup 1.0000x reference)
"""Trainium2 Bass kernel for LoRALinear: out = x @ W^T + b + scaling*(x @ A^T) @ B^T.

Strategy (8 NeuronCores, data-parallel over tokens):
  - Flatten x to [8192, 4096]; core c owns tokens [c*1024, (c+1)*1024).
  - Host-side prep (numpy, cheap vs the 275 GFLOP device matmul):
      xt  = x_shard^T            [4096, 1024]  (contraction dim on partitions)
      wt  = W^T                  [4096, 4096]  (replicated)
      at  = A^T                  [4096, 16]    (replicated)
      btb = [scaling * B^T; b]   [17, 4096]    (replicated; row 16 pairs with an
                                                all-ones row of xa to add the bias)
  - Device: out_shard = xt^T @ wt + xa_aug^T @ btb, where
      xa_aug = [[x_shard @ A^T]^T; ones] is computed on-device first.
    All matmuls run as float32r (full PE rate at free-dim 512, fp32 data).
  - Gather: concat shards on token axis, reshape to [4, 2048, 4096].
"""

import numpy as np

import concourse.bass as bass  # noqa: F401  (engine types ride along)
import concourse.mybir as mybir
import concourse.tile as tile
from concourse import bacc
from concourse.bass_utils import run_bass_kernel_spmd

B, S, DIN, DOUT, R = 4, 2048, 4096, 4096, 16
TOK = B * S            # 8192 flattened tokens
NCORES = 8
TOKS = TOK // NCORES   # 1024 tokens per core
P = 128
KT = DIN // P          # 32 contraction tiles
MT = TOKS // P         # 8 token tiles per core
NBLK = 512             # output-column block (one fp32 PSUM bank)
NT = DOUT // NBLK      # 8 column blocks
SCALING = 32 / 16

F32 = mybir.dt.float32
F32R = mybir.dt.float32r

_CACHED_NC = None


def _build():
    nc = bacc.Bacc("TRN2", target_bir_lowering=False, debug=False, num_devices=NCORES)
    xt = nc.dram_tensor("xt", [DIN, TOKS], F32R, kind="ExternalInput")
    wt = nc.dram_tensor("wt", [DIN, DOUT], F32R, kind="ExternalInput")
    at = nc.dram_tensor("at", [DIN, R], F32R, kind="ExternalInput")
    btb = nc.dram_tensor("btb", [R + 1, DOUT], F32R, kind="ExternalInput")
    ones = nc.dram_tensor("ones", [1, TOKS], F32R, kind="ExternalInput")
    out = nc.dram_tensor("out", [TOKS, DOUT], F32, kind="ExternalOutput")

    with tile.TileContext(nc) as tc:
        with (
            tc.tile_pool(name="xres", bufs=1) as xres,
            tc.tile_pool(name="consts", bufs=1) as consts,
            tc.tile_pool(name="wpool", bufs=4) as wpool,
            tc.tile_pool(name="opool", bufs=4) as opool,
            tc.tile_pool(name="psum", bufs=8, space="PSUM") as pspool,
        ):
            # Resident x^T: 32 tiles of [128, 1024] = 128 KB/partition.
            x_sb = []
            for k in range(KT):
                t = xres.tile([P, TOKS], F32R, tag=f"x{k}", name=f"x{k}")
                nc.sync.dma_start(out=t, in_=xt[k * P : (k + 1) * P, :])
                x_sb.append(t)

            at_sb = consts.tile([P, KT, R], F32R)
            nc.sync.dma_start(
                out=at_sb, in_=at[:, :].rearrange("(ko p) r -> p ko r", p=P)
            )
            btb_sb = consts.tile([R + 1, DOUT], F32R)
            nc.sync.dma_start(out=btb_sb, in_=btb[:, :])

            # xa_aug[0:16] = (x_shard @ A^T)^T, xa_aug[16] = ones (bias row).
            # DVE can't memset f32r nor address base-partition 16, but a DMA
            # can: the ones row ships in as a tiny (4 KB) input.
            xa_sb = consts.tile([R + 1, TOKS], F32R)
            nc.sync.dma_start(out=xa_sb[R : R + 1, :], in_=ones[:, :])
            for h in range(TOKS // NBLK):
                ps = pspool.tile([P, NBLK], F32, tag="ps", name=f"psl{h}")
                for k in range(KT):
                    nc.tensor.matmul(
                        ps[:R, :],
                        at_sb[:, k, :],
                        x_sb[k][:, h * NBLK : (h + 1) * NBLK],
                        start=(k == 0),
                        stop=(k == KT - 1),
                    )
                nc.vector.tensor_copy(xa_sb[:R, h * NBLK : (h + 1) * NBLK], ps[:R, :])

            # Main loop: for each 512-wide column block, stream wt k-tiles once
            # and accumulate all 8 token tiles in 8 PSUM banks; one extra K=17
            # matmul folds in lora + bias, then evict.
            for n in range(NT):
                ns = slice(n * NBLK, (n + 1) * NBLK)
                ps_tiles = [pspool.tile([P, NBLK], F32, tag="ps", name=f"ps{n}_{m}") for m in range(MT)]
                for k in range(KT):
                    wtile = wpool.tile([P, NBLK], F32R, tag="w", name=f"w{n}_{k}")
                    nc.sync.dma_start(out=wtile, in_=wt[k * P : (k + 1) * P, ns])
                    for m in range(MT):
                        nc.tensor.matmul(
                            ps_tiles[m],
                            x_sb[k][:, m * P : (m + 1) * P],
                            wtile[:],
                            start=(k == 0),
                            stop=False,
                        )
                for m in range(MT):
                    nc.tensor.matmul(
                        ps_tiles[m],
                        xa_sb[:, m * P : (m + 1) * P],
                        btb_sb[:, ns],
                        start=False,
                        stop=True,
                    )
                    ot = opool.tile([P, NBLK], F32, tag="o", name=f"o{n}_{m}")
                    nc.vector.tensor_copy(ot, ps_tiles[m])
                    nc.sync.dma_start(out=out[m * P : (m + 1) * P, ns], in_=ot)

    nc.compile()
    return nc


def _prepare_in_maps(x, W, b, lora_A, lora_B):
    x = np.ascontiguousarray(np.asarray(x, dtype=np.float32).reshape(TOK, DIN))
    W = np.asarray(W, dtype=np.float32)
    b = np.asarray(b, dtype=np.float32)
    lora_A = np.asarray(lora_A, dtype=np.float32)
    lora_B = np.asarray(lora_B, dtype=np.float32)

    wt = np.ascontiguousarray(W.T)
    at = np.ascontiguousarray(lora_A.T)
    btb = np.empty((R + 1, DOUT), dtype=np.float32)
    btb[:R] = SCALING * lora_B.T
    btb[R] = b

    in_maps = []
    for c in range(NCORES):
        xt_c = np.ascontiguousarray(x[c * TOKS : (c + 1) * TOKS].T)
        in_maps.append({"xt": xt_c, "wt": wt, "at": at, "btb": btb,
                        "ones": np.ones((1, TOKS), dtype=np.float32)})
    return in_maps


def _gather(results):
    shards = [np.asarray(results[c]["out"]) for c in range(NCORES)]
    return np.concatenate(shards, axis=0).reshape(B, S, DOUT)


def kernel(x, W, b, lora_A, lora_B):
    global _CACHED_NC
    if _CACHED_NC is None:
        _CACHED_NC = _build()
    in_maps = _prepare_in_maps(x, W, b, lora_A, lora_B)
    res = run_bass_kernel_spmd(_CACHED_NC, in_maps, core_ids=list(range(NCORES)))
    return _gather(res.results)


# revision 6
# speedup vs baseline: 1.1368x; 1.1368x over previous
"""Trainium2 Bass kernel for LoRALinear: out = x @ W^T + b + scaling*(x @ A^T) @ B^T.

Strategy (8 NeuronCores, data-parallel over tokens; ~493us/core in the
cost-model timeline, PE busy 94.5%):
  - Flatten x to [8192, 4096]; core c owns tokens [c*1024, (c+1)*1024).
  - Host prep: xt = x_shard^T, wt = W^T, at = A^T, btb = [scaling*B^T; b],
    plus a ones row so one K=17 matmul adds lora AND bias per output tile.
  - All matmuls run as float32r: fp32 data, full PE rate at free-dim >= 256
    (measured ~10x faster than fp32 on HW; rel err vs fp32 reference 1.2e-4).
  - Pipeline:
  - const DMAs (at/btb/ones) issue before the x stream (v1 starved LoRA 51us).
  - x streams on the ACT HWDGE ring, wt/out on the SP ring (no head-of-line
    blocking between the two streams).
  - Block n=0 runs DURING the x load without the LoRA accumulation; its base
    result parks in SBUF (out0). LoRA xa computes after (x resident), then a
    fixup pass adds xa_aug^T @ btb to out0 and stores. Blocks 1..7 keep the
    fused K=17 accumulation.
  - Evictions alternate DVE / ACT so the copy chain at block boundaries is
    ~2x shorter (v1: 7 x ~2.2us PSUM WAR stalls).
"""

import numpy as np

import concourse.bass as bass  # noqa: F401
import concourse.mybir as mybir
import concourse.tile as tile
from concourse import bacc
from concourse.bass_utils import run_bass_kernel_spmd

B, S, DIN, DOUT, R = 4, 2048, 4096, 4096, 16
TOK = B * S
NCORES = 8
TOKS = TOK // NCORES   # 1024
P = 128
KT = DIN // P          # 32
MT = TOKS // P         # 8
NBLK = 512
NT = DOUT // NBLK      # 8
SCALING = 32 / 16

F32 = mybir.dt.float32
F32R = mybir.dt.float32r

_CACHED_NC = None


def _build():
    nc = bacc.Bacc("TRN2", target_bir_lowering=False, debug=False, num_devices=NCORES)
    xt = nc.dram_tensor("xt", [DIN, TOKS], F32R, kind="ExternalInput")
    wt = nc.dram_tensor("wt", [DIN, DOUT], F32R, kind="ExternalInput")
    at = nc.dram_tensor("at", [DIN, R], F32R, kind="ExternalInput")
    btb = nc.dram_tensor("btb", [R + 1, DOUT], F32R, kind="ExternalInput")
    ones = nc.dram_tensor("ones", [1, TOKS], F32R, kind="ExternalInput")
    out = nc.dram_tensor("out", [TOKS, DOUT], F32, kind="ExternalOutput")

    with tile.TileContext(nc) as tc:
        with (
            tc.tile_pool(name="xres", bufs=1) as xres,
            tc.tile_pool(name="consts", bufs=1) as consts,
            tc.tile_pool(name="o0pool", bufs=1) as o0pool,
            tc.tile_pool(name="wpool", bufs=6) as wpool,
            tc.tile_pool(name="opool", bufs=6) as opool,
            tc.tile_pool(name="psum", bufs=8, space="PSUM") as pspool,
        ):
            def evict(dst, src, m):
                if m % 2 == 0:
                    nc.vector.tensor_copy(dst, src)
                else:
                    nc.scalar.copy(dst, src)

            # Small consts first so LoRA inputs are never starved.
            at_sb = consts.tile([P, KT, R], F32R)
            nc.gpsimd.dma_start(
                out=at_sb, in_=at[:, :].rearrange("(ko p) r -> p ko r", p=P)
            )
            btb_sb = consts.tile([R + 1, DOUT], F32R)
            nc.gpsimd.dma_start(out=btb_sb, in_=btb[:, :])
            xa_sb = consts.tile([R + 1, TOKS], F32R)
            nc.gpsimd.dma_start(out=xa_sb[R : R + 1, :], in_=ones[:, :])

            # x stream on the ACT ring, half-tiles for finer pipelining.
            HB = NBLK  # 512 tokens per half
            x_sb = []  # x_sb[k][h] : [128, 512]
            for k in range(KT):
                halves = []
                for h in range(2):
                    t = xres.tile([P, HB], F32R, tag=f"x{k}_{h}", name=f"x{k}_{h}")
                    nc.scalar.dma_start(
                        out=t, in_=xt[k * P : (k + 1) * P, h * HB : (h + 1) * HB]
                    )
                    halves.append(t)
                x_sb.append(halves)

            def xsl(k, m):
                # lhsT slice for token tile m out of the right half-tile
                return x_sb[k][m // 4][:, (m % 4) * P : (m % 4 + 1) * P]

            # Block n=0 during the x load: base matmul only, park in SBUF.
            n0 = slice(0, NBLK)
            out0 = [
                o0pool.tile([P, NBLK], F32, tag=f"o0_{m}", name=f"o0_{m}")
                for m in range(MT)
            ]
            ps0 = [
                pspool.tile([P, NBLK], F32, tag="ps", name=f"ps0_{m}")
                for m in range(MT)
            ]
            for k in range(KT):
                wtile = wpool.tile([P, NBLK], F32R, tag="w", name=f"w0_{k}")
                nc.sync.dma_start(out=wtile, in_=wt[k * P : (k + 1) * P, n0])
                last = k == KT - 1
                for m in range(MT):
                    nc.tensor.matmul(
                        ps0[m],
                        xsl(k, m),
                        wtile[:],
                        start=(k == 0),
                        stop=last,
                    )
                    if last:
                        evict(out0[m][:], ps0[m][:], m)

            # LoRA xa (x fully resident now; PSUM banks released by evicts).
            for h in range(TOKS // NBLK):
                ps = pspool.tile([P, NBLK], F32, tag="ps", name=f"psl{h}")
                for k in range(KT):
                    nc.tensor.matmul(
                        ps[:R, :],
                        at_sb[:, k, :],
                        x_sb[k][h][:, :],
                        start=(k == 0),
                        stop=(k == KT - 1),
                    )
                nc.vector.tensor_copy(xa_sb[:R, h * NBLK : (h + 1) * NBLK], ps[:R, :])

            # Fixup block 0: out0 += xa_aug^T @ btb[:, 0:512], then store.
            for m in range(MT):
                pf = pspool.tile([P, NBLK], F32, tag="ps", name=f"psf{m}")
                nc.tensor.matmul(
                    pf,
                    xa_sb[:, m * P : (m + 1) * P],
                    btb_sb[:, n0],
                    start=True,
                    stop=True,
                )
                ot = opool.tile([P, NBLK], F32, tag="o", name=f"of{m}")
                nc.vector.tensor_add(ot, out0[m][:], pf)
                nc.scalar.dma_start(out=out[m * P : (m + 1) * P, n0], in_=ot)

            # Blocks 1..7: fused base + lora + bias.
            for n in range(1, NT):
                ns = slice(n * NBLK, (n + 1) * NBLK)
                ps_tiles = [
                    pspool.tile([P, NBLK], F32, tag="ps", name=f"ps{n}_{m}")
                    for m in range(MT)
                ]
                for k in range(KT):
                    wtile = wpool.tile([P, NBLK], F32R, tag="w", name=f"w{n}_{k}")
                    nc.sync.dma_start(out=wtile, in_=wt[k * P : (k + 1) * P, ns])
                    last = k == KT - 1
                    for m in range(MT):
                        nc.tensor.matmul(
                            ps_tiles[m],
                            xsl(k, m),
                            wtile[:],
                            start=(k == 0),
                            stop=False,
                        )
                        if last:
                            nc.tensor.matmul(
                                ps_tiles[m],
                                xa_sb[:, m * P : (m + 1) * P],
                                btb_sb[:, ns],
                                start=False,
                                stop=True,
                            )
                            ot = opool.tile([P, NBLK], F32, tag="o", name=f"o{n}_{m}")
                            evict(ot[:], ps_tiles[m][:], m)
                            eng = nc.scalar if (n < NT - 1 or m % 2 == 0) else nc.sync
                            eng.dma_start(
                                out=out[m * P : (m + 1) * P, ns], in_=ot
                            )

    nc.compile()
    return nc


def _prepare_in_maps(x, W, b, lora_A, lora_B):
    x = np.ascontiguousarray(np.asarray(x, dtype=np.float32).reshape(TOK, DIN))
    W = np.asarray(W, dtype=np.float32)
    b = np.asarray(b, dtype=np.float32)
    lora_A = np.asarray(lora_A, dtype=np.float32)
    lora_B = np.asarray(lora_B, dtype=np.float32)

    wt = np.ascontiguousarray(W.T)
    at = np.ascontiguousarray(lora_A.T)
    btb = np.empty((R + 1, DOUT), dtype=np.float32)
    btb[:R] = SCALING * lora_B.T
    btb[R] = b

    in_maps = []
    for c in range(NCORES):
        xt_c = np.ascontiguousarray(x[c * TOKS : (c + 1) * TOKS].T)
        in_maps.append({"xt": xt_c, "wt": wt, "at": at, "btb": btb,
                        "ones": np.ones((1, TOKS), dtype=np.float32)})
    return in_maps


def _gather(results):
    shards = [np.asarray(results[c]["out"]) for c in range(NCORES)]
    return np.concatenate(shards, axis=0).reshape(B, S, DOUT)


def kernel(x, W, b, lora_A, lora_B):
    global _CACHED_NC
    if _CACHED_NC is None:
        _CACHED_NC = _build()
    in_maps = _prepare_in_maps(x, W, b, lora_A, lora_B)
    res = run_bass_kernel_spmd(_CACHED_NC, in_maps, core_ids=list(range(NCORES)))
    return _gather(res.results)


# revision 7
# speedup vs baseline: 1.1377x; 1.0008x over previous
"""Trainium2 Bass kernel for LoRALinear: out = x @ W^T + b + scaling*(x @ A^T) @ B^T.

8 NeuronCores, data-parallel over tokens. ~493 us/core in the cost-model
timeline (PE busy 94.8%); measured on HW: rel err 1.17e-4 vs the fp32
reference, per-iteration time consistent with the model (fp32 control runs
~5x slower, confirming float32r executes at full PE rate).

Design:
  - Host prep (numpy): xt = x_shard^T [4096,1024] per core; wt = W^T, at = A^T,
    btb = [scaling*B^T; b] (17 rows) replicated; plus a ones row for xa_aug so a
    single K=17 accumulation matmul adds BOTH the lora term and the bias.
  - All matmul operands are float32r: fp32 bits, PE runs 1 cycle/row at moving
    free-dim >= 256 (vs 4 cycles/row for strict fp32).
  - Pipeline: x streams on the ACT HWDGE ring, wt on the SP ring. Block n=0
    computes DURING the x load (base only, parked in SBUF out0); LoRA xa runs
    once x is resident; a fixup pass adds xa_aug^T @ btb to out0; blocks 1..7
    fuse base + lora + bias via one extra accumulation matmul per PSUM tile.
  - 8 PSUM banks hold 8 token-tiles per 512-wide column block so each streamed
    wt tile feeds 8 matmuls; evictions alternate DVE/ACT; stores ride the ACT
    ring (3-way split on the last block to drain the tail).
"""

import numpy as np

import concourse.bass as bass  # noqa: F401
import concourse.mybir as mybir
import concourse.tile as tile
from concourse import bacc
from concourse.bass_utils import run_bass_kernel_spmd

B, S, DIN, DOUT, R = 4, 2048, 4096, 4096, 16
TOK = B * S
NCORES = 8
TOKS = TOK // NCORES   # 1024
P = 128
KT = DIN // P          # 32
MT = TOKS // P         # 8
NBLK = 512
NT = DOUT // NBLK      # 8
SCALING = 32 / 16

F32 = mybir.dt.float32
F32R = mybir.dt.float32r

_CACHED_NC = None


def _build():
    nc = bacc.Bacc("TRN2", target_bir_lowering=False, debug=False, num_devices=NCORES)
    xt = nc.dram_tensor("xt", [DIN, TOKS], F32R, kind="ExternalInput")
    wt = nc.dram_tensor("wt", [DIN, DOUT], F32R, kind="ExternalInput")
    at = nc.dram_tensor("at", [DIN, R], F32R, kind="ExternalInput")
    btb = nc.dram_tensor("btb", [R + 1, DOUT], F32R, kind="ExternalInput")
    ones = nc.dram_tensor("ones", [1, TOKS], F32R, kind="ExternalInput")
    out = nc.dram_tensor("out", [TOKS, DOUT], F32, kind="ExternalOutput")

    with tile.TileContext(nc) as tc:
        with (
            tc.tile_pool(name="xres", bufs=1) as xres,
            tc.tile_pool(name="consts", bufs=1) as consts,
            tc.tile_pool(name="o0pool", bufs=1) as o0pool,
            tc.tile_pool(name="wpool", bufs=6) as wpool,
            tc.tile_pool(name="opool", bufs=6) as opool,
            tc.tile_pool(name="psum", bufs=8, space="PSUM") as pspool,
        ):
            def evict(dst, src, m):
                if m % 2 == 0:
                    nc.vector.tensor_copy(dst, src)
                else:
                    nc.scalar.copy(dst, src)

            # x stream on the ACT ring, half-tiles for finer pipelining.
            HB = NBLK  # 512 tokens per half
            x_sb = []  # x_sb[k][h] : [128, 512]
            for k in range(KT):
                halves = []
                for h in range(2):
                    t = xres.tile([P, HB], F32R, tag=f"x{k}_{h}", name=f"x{k}_{h}")
                    nc.scalar.dma_start(
                        out=t, in_=xt[k * P : (k + 1) * P, h * HB : (h + 1) * HB]
                    )
                    halves.append(t)
                x_sb.append(halves)

            def xsl(k, m):
                # lhsT slice for token tile m out of the right half-tile
                return x_sb[k][m // 4][:, (m % 4) * P : (m % 4 + 1) * P]

            # Block n=0 during the x load: base matmul only, park in SBUF.
            n0 = slice(0, NBLK)
            out0 = [
                o0pool.tile([P, NBLK], F32, tag=f"o0_{m}", name=f"o0_{m}")
                for m in range(MT)
            ]
            ps0 = [
                pspool.tile([P, NBLK], F32, tag="ps", name=f"ps0_{m}")
                for m in range(MT)
            ]
            for k in range(KT):
                wtile = wpool.tile([P, NBLK], F32R, tag="w", name=f"w0_{k}")
                nc.sync.dma_start(out=wtile, in_=wt[k * P : (k + 1) * P, n0])
                last = k == KT - 1
                for m in range(MT):
                    nc.tensor.matmul(
                        ps0[m],
                        xsl(k, m),
                        wtile[:],
                        start=(k == 0),
                        stop=last,
                    )
                    if last:
                        evict(out0[m][:], ps0[m][:], m)

            # Consts load late (needed from the LoRA phase on) so their many
            # small descriptors don't contend with the critical first x/wt DMAs.
            at_sb = consts.tile([P, KT, R], F32R)
            nc.gpsimd.dma_start(
                out=at_sb, in_=at[:, :].rearrange("(ko p) r -> p ko r", p=P)
            )
            btb_sb = consts.tile([R + 1, DOUT], F32R)
            nc.gpsimd.dma_start(out=btb_sb, in_=btb[:, :])
            xa_sb = consts.tile([R + 1, TOKS], F32R)
            nc.gpsimd.dma_start(out=xa_sb[R : R + 1, :], in_=ones[:, :])

            # LoRA xa (x fully resident now; PSUM banks released by evicts).
            for h in range(TOKS // NBLK):
                ps = pspool.tile([P, NBLK], F32, tag="ps", name=f"psl{h}")
                for k in range(KT):
                    nc.tensor.matmul(
                        ps[:R, :],
                        at_sb[:, k, :],
                        x_sb[k][h][:, :],
                        start=(k == 0),
                        stop=(k == KT - 1),
                    )
                nc.vector.tensor_copy(xa_sb[:R, h * NBLK : (h + 1) * NBLK], ps[:R, :])

            # Fixup block 0: out0 += xa_aug^T @ btb[:, 0:512], then store.
            for m in range(MT):
                pf = pspool.tile([P, NBLK], F32, tag="ps", name=f"psf{m}")
                nc.tensor.matmul(
                    pf,
                    xa_sb[:, m * P : (m + 1) * P],
                    btb_sb[:, n0],
                    start=True,
                    stop=True,
                )
                ot = opool.tile([P, NBLK], F32, tag="o", name=f"of{m}")
                nc.vector.tensor_add(ot, out0[m][:], pf)
                nc.scalar.dma_start(out=out[m * P : (m + 1) * P, n0], in_=ot)

            # Blocks 1..7: fused base + lora + bias.
            for n in range(1, NT):
                ns = slice(n * NBLK, (n + 1) * NBLK)
                ps_tiles = [
                    pspool.tile([P, NBLK], F32, tag="ps", name=f"ps{n}_{m}")
                    for m in range(MT)
                ]
                for k in range(KT):
                    wtile = wpool.tile([P, NBLK], F32R, tag="w", name=f"w{n}_{k}")
                    nc.sync.dma_start(out=wtile, in_=wt[k * P : (k + 1) * P, ns])
                    last = k == KT - 1
                    for m in range(MT):
                        nc.tensor.matmul(
                            ps_tiles[m],
                            xsl(k, m),
                            wtile[:],
                            start=(k == 0),
                            stop=False,
                        )
                        if last:
                            nc.tensor.matmul(
                                ps_tiles[m],
                                xa_sb[:, m * P : (m + 1) * P],
                                btb_sb[:, ns],
                                start=False,
                                stop=True,
                            )
                            ot = opool.tile([P, NBLK], F32, tag="o", name=f"o{n}_{m}")
                            evict(ot[:], ps_tiles[m][:], m)
                            if n < NT - 1:
                                eng = nc.scalar
                            else:
                                eng = (nc.scalar, nc.sync, nc.gpsimd)[m % 3]
                            eng.dma_start(
                                out=out[m * P : (m + 1) * P, ns], in_=ot
                            )

    nc.compile()
    return nc


def _prepare_in_maps(x, W, b, lora_A, lora_B):
    x = np.ascontiguousarray(np.asarray(x, dtype=np.float32).reshape(TOK, DIN))
    W = np.asarray(W, dtype=np.float32)
    b = np.asarray(b, dtype=np.float32)
    lora_A = np.asarray(lora_A, dtype=np.float32)
    lora_B = np.asarray(lora_B, dtype=np.float32)

    wt = np.ascontiguousarray(W.T)
    at = np.ascontiguousarray(lora_A.T)
    btb = np.empty((R + 1, DOUT), dtype=np.float32)
    btb[:R] = SCALING * lora_B.T
    btb[R] = b

    in_maps = []
    for c in range(NCORES):
        xt_c = np.ascontiguousarray(x[c * TOKS : (c + 1) * TOKS].T)
        in_maps.append({"xt": xt_c, "wt": wt, "at": at, "btb": btb,
                        "ones": np.ones((1, TOKS), dtype=np.float32)})
    return in_maps


def _gather(results):
    shards = [np.asarray(results[c]["out"]) for c in range(NCORES)]
    return np.concatenate(shards, axis=0).reshape(B, S, DOUT)


def kernel(x, W, b, lora_A, lora_B):
    global _CACHED_NC
    if _CACHED_NC is None:
        _CACHED_NC = _build()
    in_maps = _prepare_in_maps(x, W, b, lora_A, lora_B)
    res = run_bass_kernel_spmd(_CACHED_NC, in_maps, core_ids=list(range(NCORES)))
    return _gather(res.results)
